# revision 57
# baseline (speedup 1.0000x reference)
"""Trainium2 Bass kernel for sliding-window causal MHA with RoPE + ALiBi.

Sharding: 8 cores = 4 batches x 2 head-groups (8 heads each).

v2: fp8 DoubleRow matmuls with 3-term hi/lo error compensation for the
q/k/v projections and the output projection (host-side hi/lo splits of x
and all weights; device-side hi/lo of the attention output).  Scores,
PV, and sums matmuls stay bf16 (fp8 there fails the accuracy gate).

Per-core program:
  A: v-proj -> v bf16 [t,hd];  q/k-proj -> rope (bf16) -> qr/kr [d,t]
  B: per head, per 256-query group: transposed scores sT[j,i] (bf16)
     -> exp (Act, psum->bf16) -> *expb mask (DVE) -> PV + ones-sums
     -> normalize -> ats hi/lo fp8
  C: out-proj 3-term hi/lo fp8 DR, partials summed on host.
"""
import sys
sys.path.insert(0, '/opt/trn_rl_repo')

import numpy as np
import ml_dtypes
import concourse.bass as bass
import concourse.bacc as bacc
import concourse.mybir as mybir
import concourse.tile as tile

L, N, C, H, D, W = 1024, 4, 2048, 16, 128, 512
HPC = 8                       # heads per core
GD = HPC * D                  # 1024 head-dims per core
SCALE = 1.0 / float(np.sqrt(D))
WS = 32.0                     # weight pre-scale before fp8 (undone at evac)
F32 = mybir.dt.float32
F8 = mybir.dt.float8e4
BF16 = mybir.dt.bfloat16
AF = mybir.ActivationFunctionType
DR = mybir.MatmulPerfMode.DoubleRow
NT_C = C // 128               # 16 contraction tiles over embed dim
NT_HD = GD // 128             # 8 head tiles (1 head each, D=128)
NT_T = L // 128               # 8 token tiles
QG = 256                      # query-group width
NQG = L // QG                 # 4
USE_DIVIDE = False            # verifier: only one PSUM input per DVE op
EW = 896                      # expb master width
MC0 = 128                     # expb center offset


# Heads are assigned to cores in slope-paired order: core group g holds
# global heads [g + 2s for s in 0..7], so SPMD slot s sees ALiBi slopes
# 2^-(2s+g+1)/2 on both cores.  Beyond ~30 nats of ALiBi decay a key tile
# contributes < 1e-9 of the softmax mass, so slot s only needs window
# W_SLOT[s] = min(512, ceil(30 / slope of its shallower head)).
W_SLOT = [60, 120, 240, 480, 512, 512, 512, 512]


def CORE_HEADS(g):
    return [g + 2 * s for s in range(HPC)]


def jtiles(i0, s=None):
    w = W if s is None else W_SLOT[s]
    lo = (max(0, i0 - w) // 128) * 128
    return list(range(lo, min(i0 + QG, L) - 128 + 1, 128))


def emit(tc, t):
    nc = tc.nc
    cpool = tc.alloc_tile_pool(name="const", bufs=1, side="left")
    cos2 = cpool.tile([128, L], BF16, tag="cos2")
    sin2 = cpool.tile([128, L], BF16, tag="sin2")
    ones = cpool.tile([128, 128], BF16, tag="ones")

    # long-lived activations (left stack)
    pp = tc.alloc_tile_pool(name="persist", bufs=1, side="left")
    qr = pp.tile([128, NT_HD, L], BF16, tag="qr", name="qr")
    kr = pp.tile([128, NT_HD, L], BF16, tag="kr", name="kr")
    vv = pp.tile([128, NT_T, GD], BF16, tag="vv", name="vv")
    ahi = pp.tile([128, HPC, L], F8, tag="ahi", name="ahi")
    alo = pp.tile([128, HPC, L], F8, tag="alo", name="alo")

    # ---------------- phase A ----------------
    # DMA order tuned so the first q/k psum (t 0:512) can start ~6us in:
    # x t-halves first, then m=0/1 q/k weights, then the rest.
    xw = tc.alloc_tile_pool(name="xw", bufs=1, side="left")
    xhi = xw.tile([128, NT_C, L], F8, tag="xhi", name="xhi")
    xlo = xw.tile([128, NT_C, L], F8, tag="xlo", name="xlo")

    def load_qk_w(wsp, m):
        wt = wsp.tile([128, 4, NT_C, 128], F8, tag="wqk", name="wqk")
        nc.sync.dma_start(wt[:], t["wqk"][m])
        return {"wqhi": wt[:, 0], "wqlo": wt[:, 1],
                "wkhi": wt[:, 2], "wklo": wt[:, 3]}

    with tc.tile_pool(name="ws", bufs=2, side="right") as wsp, \
         tc.tile_pool(name="rp", bufs=3, side="right") as rp:
        paq = pav = None  # PSUM pools created after the m=0 prologue frees its banks
        # DMA order: q weights for m=0, xhi, k weights, xlo — paired with
        # deferred emission of the xlo terms for m=0 below, PE starts as
        # soon as xhi lands and never blocks on xlo.
        nc.sync.dma_start(xhi[:, 0:8], t["xhi"][:, 0:8])
        # m=0 weights split so the q half lands with the first x chunk
        wt0 = wsp.tile([128, 4, NT_C, 128], F8, tag="wqk", name="wqk")
        nc.sync.dma_start(wt0[:, 0:2], t["wqk"][0][:, 0:2])
        nc.sync.dma_start(xhi[:, 8:16], t["xhi"][:, 8:16])
        nc.sync.dma_start(wt0[:, 2:4], t["wqk"][0][:, 2:4])
        wts_cur = {"wqhi": wt0[:, 0], "wqlo": wt0[:, 1],
                   "wkhi": wt0[:, 2], "wklo": wt0[:, 3]}
        nc.sync.dma_start(xlo[:, 0:8], t["xlo"][:, 0:8])
        nc.sync.dma_start(xlo[:, 8:16], t["xlo"][:, 8:16])
        nc.sync.dma_start(cos2[:], t["cos2"][:])
        nc.sync.dma_start(sin2[:], t["sin2"][:])
        wvhi = xw.tile([128, NT_C, GD], F8, tag="wvhi", name="wvhi")
        wvlo = xw.tile([128, NT_C, GD], F8, tag="wvlo", name="wvlo")
        wv_issued = False

        # q/k-proj: psum [d, t] = sum_c w[c,d].T @ x[c,t], rope after evac
        def qk_mms(ps, wh, wl, tg2, th, terms, start, stop):
            for xs, ws in terms:
                for a in range(NT_C // 2):
                    ts0 = th * 512 + tg2 * 256
                    nc.tensor.matmul(
                        ps[:, tg2 * 256:(tg2 + 1) * 256],
                        ws[:, 2 * a:2 * a + 2, :],
                        xs[:, 2 * a:2 * a + 2, ts0:ts0 + 256],
                        start=start, stop=stop and (xs is terms[-1][0]
                                                    and a == NT_C // 2 - 1),
                        perf_mode=DR)
                    start = False

        def rope(ps, dst, m, th):
            csl = slice(th * 512, (th + 1) * 512)
            qb = rp.tile([128, 512], BF16, tag="qb", name="qb")
            nc.scalar.mul(qb[:], ps[:], 1.0 / WS)
            t1 = rp.tile([128, 512], BF16, tag="t1", name="t1")
            nc.vector.tensor_mul(t1[:], qb[:], cos2[:, csl])
            t2 = rp.tile([128, 512], BF16, tag="t2", name="t2")
            nc.vector.tensor_mul(t2[0:64, :], qb[64:128, :],
                                 sin2[64:128, csl])
            nc.vector.tensor_mul(t2[64:128, :], qb[0:64, :],
                                 sin2[0:64, csl])
            nc.vector.tensor_add(dst[:, m, csl], t1[:], t2[:])

        if True:
            for m in range(NT_HD):
                wts = wts_cur
                if m + 1 < NT_HD:
                    wts_cur = load_qk_w(wsp, m + 1)
                if m == 0:
                    # deferred-xlo emission: all 8 psum groups get their own
                    # bank (a bank can hold only ONE open accumulation group),
                    # xhi terms first, xlo terms + stop once xlo has landed
                    paq0 = tc.alloc_tile_pool(name="paq0", bufs=1,
                                              space="PSUM")
                    pss = {}
                    started = set()
                    # q before k (k weights arrive later), ctile-half-major
                    # so the first groups only need the first xhi DMA chunk
                    for pref in ("wq", "wk"):
                        for ah in range(2):
                            wh, wl = wts[pref + "hi"], wts[pref + "lo"]
                            for th in range(2):
                                for tg2 in range(2):
                                    key = (pref, th, tg2)
                                    if key not in pss:
                                        pss[key] = paq0.tile(
                                            [128, 256], F32,
                                            tag=f"p0{pref}{th}{tg2}",
                                            name="psAq0")
                                    ps = pss[key]
                                    ts0 = th * 512 + tg2 * 256
                                    for xs, ws in ((xhi, wh), (xhi, wl)):
                                        for a in range(ah * 4, ah * 4 + 4):
                                            nc.tensor.matmul(
                                                ps[:],
                                                ws[:, 2 * a:2 * a + 2, :],
                                                xs[:, 2 * a:2 * a + 2,
                                                   ts0:ts0 + 256],
                                                start=(key not in started),
                                                stop=False,
                                                perf_mode=DR)
                                            started.add(key)
                    for pref, dst in (("wq", qr), ("wk", kr)):
                        wh, wl = wts[pref + "hi"], wts[pref + "lo"]
                        for th in range(2):
                            for tg2 in range(2):
                                ps = pss[(pref, th, tg2)]
                                ts0 = th * 512 + tg2 * 256
                                for a in range(NT_C // 2):
                                    nc.tensor.matmul(
                                        ps[:],
                                        wh[:, 2 * a:2 * a + 2, :],
                                        xlo[:, 2 * a:2 * a + 2, ts0:ts0 + 256],
                                        start=False, stop=(a == NT_C // 2 - 1),
                                        perf_mode=DR)
                                csl = slice(ts0, ts0 + 256)
                                qb = rp.tile([128, 256], BF16, tag="qb0",
                                             name="qb0")
                                nc.scalar.mul(qb[:], ps[:], 1.0 / WS)
                                t1 = rp.tile([128, 256], BF16, tag="t10",
                                             name="t10")
                                nc.vector.tensor_mul(t1[:], qb[:],
                                                     cos2[:, csl])
                                t2 = rp.tile([128, 256], BF16, tag="t20",
                                             name="t20")
                                nc.vector.tensor_mul(t2[0:64, :],
                                                     qb[64:128, :],
                                                     sin2[64:128, csl])
                                nc.vector.tensor_mul(t2[64:128, :],
                                                     qb[0:64, :],
                                                     sin2[0:64, csl])
                                nc.vector.tensor_add(dst[:, m, csl],
                                                     t1[:], t2[:])
                    paq0.release()
                    paq = tc.alloc_tile_pool(name="paq", bufs=2, space="PSUM")
                    pav = tc.alloc_tile_pool(name="pav", bufs=2, space="PSUM")
                    continue
                if not wv_issued:
                    # ones (first needed by the first sums matmul in phase B)
                    # and wv queue behind the m=1 weights
                    nc.sync.dma_start(ones[:], t["ones"][:])
                    nc.sync.dma_start(wvhi[:], t["wvhi"][:])
                    nc.sync.dma_start(wvlo[:], t["wvlo"][:])
                    wv_issued = True
                for pref, dst in (("wq", qr), ("wk", kr)):
                    wh, wl = wts[pref + "hi"], wts[pref + "lo"]
                    for th in range(2):
                        ps = paq.tile([128, 512], F32, tag="pq", name="psAq")
                        for tg2 in range(2):
                            qk_mms(ps, wh, wl, tg2, th,
                                   ((xhi, wh), (xhi, wl), (xlo, wh)),
                                   tg2 is not None and True, True)
                        rope(ps, dst, m, th)

        # v-proj: psum [t, hd] = sum_c x[c,t].T @ wv[c,hd], 3-term hi/lo DR
        for tt in range(NT_T):
            tsl = slice(tt * 128, (tt + 1) * 128)
            for half in range(2):
                ps = pav.tile([128, 512], F32, tag="pv", name="psAv")
                for hb2 in range(2):
                    hb = half * 512 + hb2 * 256
                    idx = 0
                    for xs, ws in ((xhi, wvhi), (xhi, wvlo), (xlo, wvhi)):
                        for a in range(NT_C // 2):
                            nc.tensor.matmul(
                                ps[:, hb2 * 256:(hb2 + 1) * 256],
                                xs[:, 2 * a:2 * a + 2, tsl],
                                ws[:, 2 * a:2 * a + 2, hb:hb + 256],
                                start=(idx == 0), stop=(idx == 23),
                                perf_mode=DR)
                            idx += 1
                nc.scalar.mul(vv[:, tt, half * 512:(half + 1) * 512],
                              ps[:], 1.0 / WS)
        pav.release()
        paq.release()

    xw.release()

    wop = tc.alloc_tile_pool(name="wop", bufs=1, side="left")
    wohi = wop.tile([128, NT_HD, C], F8, tag="wohi", name="wohi")
    wolo = wop.tile([128, NT_HD, C], F8, tag="wolo", name="wolo")

    # ---------------- phase B: attention ----------------
    with tc.tile_pool(name="mp", bufs=HPC, side="right") as mp, \
         tc.tile_pool(name="ep", bufs=6, side="right") as ep, \
         tc.tile_pool(name="ptp", bufs=16, side="right") as ptp, \
         tc.tile_pool(name="nrm", bufs=4, side="right") as nrm, \
         tc.tile_pool(name="pcs", bufs=4, space="PSUM") as pcs, \
         tc.tile_pool(name="pca", bufs=2, space="PSUM") as pca, \
         tc.tile_pool(name="pcm", bufs=2, space="PSUM") as pcm:
        # all expb masks up front, then the (big) out-proj weights, so the
        # per-head mask is never behind a 4MB transfer on the DMA queue
        expbs = []
        for h in range(HPC):
            eb = mp.tile([128, EW], BF16, tag="expb", name="expb")
            nc.sync.dma_start(eb[:], t["expb"][h])
            expbs.append(eb)
        nc.sync.dma_start(wohi[:], t["wohi"][:])
        nc.sync.dma_start(wolo[:], t["wolo"][:])
        # software pipeline: emit group k's scores/exp/mask one group ahead
        # of group k-1's PV+sums, so the in-order PE never sits in the
        # scores->exp->mask->PV latency chain (worst for the 2-tile group 0)
        def emit_sc(h, gi):
            """scores matmuls + exp; masks are emitted separately so the
            previous group's normalize is not queued behind them on DVE"""
            i0 = gi * QG
            js = jtiles(i0, h)
            scs = []
            for pi in range(0, len(js), 2):
                pair = js[pi:pi + 2]
                npc = len(pair)
                s_ps = pcs.tile([128, 512], F32, tag="s", name="s_ps")
                for k2, j0 in enumerate(pair):
                    nc.tensor.matmul(
                        s_ps[:, k2 * 256:(k2 + 1) * 256],
                        kr[:, h, j0:j0 + 128],
                        qr[:, h, i0:i0 + QG],
                        start=True, stop=True)
                e = ep.tile([128, 512], BF16, tag="e", name="e")
                nc.scalar.activation(e[:, 0:npc * 256], s_ps[:, 0:npc * 256],
                                     AF.Exp, scale=SCALE)
                scs.append((pair, e))
            return scs

        def emit_mk(h, gi, scs):
            i0 = gi * QG
            expb = expbs[h]
            pts = []
            for pair, e in scs:
                for k2, j0 in enumerate(pair):
                    soff = MC0 - (j0 - i0)
                    pT = ptp.tile([128, QG], BF16, tag="pT", name="pT")
                    # spill a fraction of the mask-muls to GPSIMD to keep
                    # DVE off the phase-B critical path
                    self_cnt = emit_mk.cnt = getattr(emit_mk, 'cnt', 0) + 1
                    eng = nc.gpsimd if self_cnt % 7 == 6 else nc.vector
                    eng.tensor_mul(
                        pT[:], e[:, k2 * 256:(k2 + 1) * 256],
                        expb[:, soff:soff + QG])
                    pts.append((j0, pT))
            return pts

        def emit_pv(h, gi, pts):
            i0 = gi * QG
            nj = len(pts)
            attn_ps = pca.tile([128, QG], F32, tag="attn", name="attn_ps")
            sums_ps = pcm.tile([128, QG], F32, tag="sums", name="sums_ps")
            for idx, (j0, pT) in enumerate(pts):
                nc.tensor.matmul(
                    attn_ps[:],
                    vv[:, j0 // 128, h * 128:(h + 1) * 128],
                    pT[:],
                    start=(idx == 0), stop=(idx == nj - 1))
                nc.tensor.matmul(
                    sums_ps[:],
                    ones[:],
                    pT[:],
                    start=(idx == 0), stop=(idx == nj - 1))
            tmp = nrm.tile([128, QG], BF16, tag="tmp", name="tmp")
            rec = nrm.tile([128, QG], F32, tag="rec", name="rec")
            nc.vector.reciprocal(rec[:], sums_ps[:])
            nc.vector.tensor_mul(tmp[:], attn_ps[:], rec[:])
            # hi/lo split spread over GPSIMD + Act (both off critical path)
            if gi % 2 == 0:
                nc.gpsimd.tensor_copy(ahi[:, h, i0:i0 + QG], tmp[:])
            else:
                nc.scalar.copy(ahi[:, h, i0:i0 + QG], tmp[:])
            nc.gpsimd.tensor_sub(alo[:, h, i0:i0 + QG], tmp[:],
                                 ahi[:, h, i0:i0 + QG])

        groups = [(h, gi) for h in range(HPC) for gi in range(NQG)]
        prev = None
        for h, gi in groups:
            scs = emit_sc(h, gi)
            pts = emit_mk(h, gi, scs)
            if prev is not None:
                emit_pv(*prev)
            prev = (h, gi, pts)
        emit_pv(*prev)

    # ---------------- phase C: out-proj (3-term hi/lo DR) ----------------
    with tc.tile_pool(name="og", bufs=4, side="right") as og, \
         tc.tile_pool(name="pd", bufs=4, space="PSUM") as pd:
        for tt in range(NT_T):
            tsl = slice(tt * 128, (tt + 1) * 128)
            for cq in range(2):
                o = og.tile([128, 1024], BF16, tag="o", name="o")
                for cb4 in range(4):
                    cb = cq * 4 + cb4
                    ps = pd.tile([128, 256], F32, tag="po", name="psD")
                    idx = 0
                    for a_, w_ in ((ahi, wohi), (alo, wohi), (ahi, wolo)):
                        for hp in range(NT_HD // 2):
                            nc.tensor.matmul(
                                ps[:],
                                a_[:, 2 * hp:2 * hp + 2, tsl],
                                w_[:, 2 * hp:2 * hp + 2,
                                   cb * 256:(cb + 1) * 256],
                                start=(idx == 0), stop=(idx == 11),
                                perf_mode=DR)
                            idx += 1
                    osl = o[:, cb4 * 256:(cb4 + 1) * 256]
                    if cb % 2 == 0:
                        nc.scalar.mul(osl, ps[:], 1.0 / WS)
                    else:
                        nc.vector.tensor_scalar_mul(osl, ps[:], 1.0 / WS)
                nc.sync.dma_start(
                    t["out"][tt * 128:(tt + 1) * 128,
                             cq * 1024:(cq + 1) * 1024], o[:])

    wop.release()
    pp.release()
    cpool.release()


def build_nc(enable_asserts=False, reps=1):
    nc = bacc.Bacc("TRN2", target_bir_lowering=False, debug=False,
                   enable_asserts=enable_asserts, num_devices=8)
    t = {}
    t["xhi"] = nc.dram_tensor("xhi", [128, NT_C, L], F8, kind="ExternalInput").ap()
    t["xlo"] = nc.dram_tensor("xlo", [128, NT_C, L], F8, kind="ExternalInput").ap()
    t["wvhi"] = nc.dram_tensor("wvhi", [128, NT_C, GD], F8, kind="ExternalInput").ap()
    t["wvlo"] = nc.dram_tensor("wvlo", [128, NT_C, GD], F8, kind="ExternalInput").ap()
    t["wqk"] = nc.dram_tensor("wqk", [NT_HD, 128, 4, NT_C, 128], F8,
                              kind="ExternalInput").ap()
    t["wohi"] = nc.dram_tensor("wohi", [128, NT_HD, C], F8, kind="ExternalInput").ap()
    t["wolo"] = nc.dram_tensor("wolo", [128, NT_HD, C], F8, kind="ExternalInput").ap()
    t["cos2"] = nc.dram_tensor("cos2", [128, L], BF16, kind="ExternalInput").ap()
    t["sin2"] = nc.dram_tensor("sin2", [128, L], BF16, kind="ExternalInput").ap()
    t["expb"] = nc.dram_tensor("expb", [HPC, 128, EW], BF16, kind="ExternalInput").ap()
    t["ones"] = nc.dram_tensor("ones", [128, 128], BF16, kind="ExternalInput").ap()
    t["out"] = nc.dram_tensor("out", [L, C], BF16, kind="ExternalOutput").ap()
    with tile.TileContext(nc) as tc:
        for _ in range(reps):
            emit(tc, t)
    nc.compile()
    return nc


def _split8(a):
    """hi/lo fp8 split of an fp32 array."""
    f8 = ml_dtypes.float8_e4m3
    hi = a.astype(f8)
    lo = (a - hi.astype(np.float32)).astype(f8)
    return hi, lo


def marshal(inputs):
    x = np.asarray(inputs["x"], np.float32)
    wq = np.asarray(inputs["wq"], np.float32)
    wkv = np.asarray(inputs["wkv"], np.float32)
    wo = np.asarray(inputs["wo"], np.float32)
    alibi = np.asarray(inputs["alibi_slopes"], np.float32)
    wk_full, wv_full = wkv[:C], wkv[C:]

    perm = np.concatenate([np.arange(0, D, 2), np.arange(1, D, 2)])
    head_perm = np.concatenate([h * D + perm for h in range(H)])
    wq_p, wk_p = wq[head_perm], wk_full[head_perm]

    t_abs = np.arange(W, W + L, dtype=np.float64)
    inv = 1.0 / (10000.0 ** (np.arange(0, D, 2, dtype=np.float64) / D))
    fr = np.outer(t_abs, inv)
    cosT = np.cos(fr).T.astype(np.float32)
    sinT = np.sin(fr).T.astype(np.float32)
    bf = ml_dtypes.bfloat16
    cos2 = np.ascontiguousarray(np.concatenate([cosT, cosT], 0)).astype(bf)
    # partition-swapped sin master: rows 0:64 = +sinT (mult for x1 -> out
    # rows 64:128), rows 64:128 = -sinT (mult for x2 -> out rows 0:64);
    # keeps both tensor_tensor inputs at the same base partition.
    sin2 = np.ascontiguousarray(np.concatenate([sinT, -sinT], 0)).astype(bf)

    # expb master: [dj, y] = exp(slope*rel) * window, rel = dj - y + MC0
    dj = np.arange(128)[:, None]
    y = np.arange(EW)[None, :]
    rel = (dj - y + MC0).astype(np.float64)
    win = (rel <= 0) & (rel >= -W)

    in_maps = []
    for core in range(8):
        b, g = divmod(core, 2)
        hsel = CORE_HEADS(g)
        rsel = np.concatenate([np.arange(hh * D, (hh + 1) * D) for hh in hsel])
        xb = x[:, b, :]                                   # (L, C)
        xT = np.ascontiguousarray(xb.T).reshape(NT_C, 128, L)
        xT = np.ascontiguousarray(xT.transpose(1, 0, 2))  # [128, NT_C, L]
        xhi, xlo = _split8(xT)
        # wv: [c-part, ctile, hd]
        wv_m = np.ascontiguousarray(
            wv_full[rsel].T.reshape(NT_C, 128, GD).transpose(1, 0, 2))
        wvhi, wvlo = _split8(wv_m * WS)
        # wq/wk: [m, c-part, ctile, d]
        def qk_m(w):
            wg = w[rsel].reshape(NT_HD, 128, NT_C, 128)   # [m, d, ct, cp]
            return np.ascontiguousarray(wg.transpose(0, 3, 2, 1))
        wqhi, wqlo = _split8(qk_m(wq_p) * WS)
        wkhi, wklo = _split8(qk_m(wk_p) * WS)
        wqk = np.ascontiguousarray(
            np.stack([wqhi, wqlo, wkhi, wklo], axis=2))
        # wo: [dv-part, hdtile, c]
        wo_m = np.ascontiguousarray(
            wo[:, rsel].T.reshape(NT_HD, 128, C).transpose(1, 0, 2))
        wohi, wolo = _split8(wo_m * WS)
        expb = np.zeros((HPC, 128, EW), bf)
        for hh in range(HPC):
            s = float(alibi[hsel[hh]])
            expb[hh] = np.where(win, np.exp(s * rel), 0.0).astype(bf)
        in_maps.append(dict(
            xhi=xhi, xlo=xlo, wvhi=wvhi, wvlo=wvlo, wqk=wqk,
            wohi=wohi, wolo=wolo,
            cos2=cos2, sin2=sin2, expb=expb,
            ones=np.ones((128, 128), bf)))
    return in_maps


def gather(results, bo):
    bo = np.asarray(bo, np.float32)
    out = np.empty((L, N, C), np.float32)
    for b in range(N):
        out[:, b, :] = (results[2 * b]["out"].astype(np.float32)
                        + results[2 * b + 1]["out"].astype(np.float32)
                        + bo[None, :])
    return out


_NC_CACHE = {}


def _get_nc():
    if "nc" not in _NC_CACHE:
        _NC_CACHE["nc"] = build_nc()
    return _NC_CACHE["nc"]


def kernel(**inputs):
    from concourse import bass_utils
    nc = _get_nc()
    in_maps = marshal(inputs)
    res = bass_utils.run_bass_kernel_spmd(nc, in_maps, core_ids=list(range(8)))
    return gather(res.results, inputs["bo"])


# revision 58
# speedup vs baseline: 1.0061x; 1.0061x over previous
"""Trainium2 Bass kernel for sliding-window causal MHA with RoPE + ALiBi.

Sharding: 8 cores = 4 batches x 2 head-groups (8 heads each).

v2: fp8 DoubleRow matmuls with 3-term hi/lo error compensation for the
q/k/v projections and the output projection (host-side hi/lo splits of x
and all weights; device-side hi/lo of the attention output).  Scores,
PV, and sums matmuls stay bf16 (fp8 there fails the accuracy gate).

Per-core program:
  A: v-proj -> v bf16 [t,hd];  q/k-proj -> rope (bf16) -> qr/kr [d,t]
  B: per head, per 256-query group: transposed scores sT[j,i] (bf16)
     -> exp (Act, psum->bf16) -> *expb mask (DVE) -> PV + ones-sums
     -> normalize -> ats hi/lo fp8
  C: out-proj 3-term hi/lo fp8 DR, partials summed on host.
"""
import sys
sys.path.insert(0, '/opt/trn_rl_repo')

import numpy as np
import ml_dtypes
import concourse.bass as bass
import concourse.bacc as bacc
import concourse.mybir as mybir
import concourse.tile as tile

L, N, C, H, D, W = 1024, 4, 2048, 16, 128, 512
HPC = 8                       # heads per core
GD = HPC * D                  # 1024 head-dims per core
SCALE = 1.0 / float(np.sqrt(D))
WS = 32.0                     # weight pre-scale before fp8 (undone at evac)
F32 = mybir.dt.float32
F8 = mybir.dt.float8e4
BF16 = mybir.dt.bfloat16
AF = mybir.ActivationFunctionType
DR = mybir.MatmulPerfMode.DoubleRow
NT_C = C // 128               # 16 contraction tiles over embed dim
NT_HD = GD // 128             # 8 head tiles (1 head each, D=128)
NT_T = L // 128               # 8 token tiles
QG = 256                      # query-group width
NQG = L // QG                 # 4
USE_DIVIDE = False            # verifier: only one PSUM input per DVE op
EW = 896                      # expb master width
MC0 = 128                     # expb center offset


# Heads are assigned to cores in slope-paired order: core group g holds
# global heads [g + 2s for s in 0..7], so SPMD slot s sees ALiBi slopes
# 2^-(2s+g+1)/2 on both cores.  Beyond ~30 nats of ALiBi decay a key tile
# contributes < 1e-9 of the softmax mass, so slot s only needs window
# W_SLOT[s] = min(512, ceil(30 / slope of its shallower head)).
W_SLOT = [60, 120, 240, 480, 512, 512, 512, 512]


def CORE_HEADS(g):
    return [g + 2 * s for s in range(HPC)]


def jtiles(i0, s=None):
    w = W if s is None else W_SLOT[s]
    lo = (max(0, i0 - w) // 128) * 128
    return list(range(lo, min(i0 + QG, L) - 128 + 1, 128))


def emit(tc, t):
    nc = tc.nc
    cpool = tc.alloc_tile_pool(name="const", bufs=1, side="left")
    cos2 = cpool.tile([128, L], BF16, tag="cos2")
    sin2 = cpool.tile([128, L], BF16, tag="sin2")
    ones = cpool.tile([128, 128], BF16, tag="ones")

    # long-lived activations (left stack)
    pp = tc.alloc_tile_pool(name="persist", bufs=1, side="left")
    qr = pp.tile([128, NT_HD, L], BF16, tag="qr", name="qr")
    kr = pp.tile([128, NT_HD, L], BF16, tag="kr", name="kr")
    vv = pp.tile([128, NT_T, GD], BF16, tag="vv", name="vv")
    ahi = pp.tile([128, HPC, L], F8, tag="ahi", name="ahi")
    alo = pp.tile([128, HPC, L], F8, tag="alo", name="alo")

    # ---------------- phase A ----------------
    # DMA order tuned so the first q/k psum (t 0:512) can start ~6us in:
    # x t-halves first, then m=0/1 q/k weights, then the rest.
    xw = tc.alloc_tile_pool(name="xw", bufs=1, side="left")
    xhi = xw.tile([128, NT_C, L], F8, tag="xhi", name="xhi")
    xlo = xw.tile([128, NT_C, L], F8, tag="xlo", name="xlo")

    def load_qk_w(wsp, m):
        wt = wsp.tile([128, 4, NT_C, 128], F8, tag="wqk", name="wqk")
        nc.sync.dma_start(wt[:], t["wqk"][m])
        return {"wqhi": wt[:, 0], "wqlo": wt[:, 1],
                "wkhi": wt[:, 2], "wklo": wt[:, 3]}

    with tc.tile_pool(name="ws", bufs=2, side="right") as wsp, \
         tc.tile_pool(name="rp", bufs=3, side="right") as rp:
        paq = pav = None  # PSUM pools created after the m=0 prologue frees its banks
        # DMA order: q weights for m=0, xhi, k weights, xlo — paired with
        # deferred emission of the xlo terms for m=0 below, PE starts as
        # soon as xhi lands and never blocks on xlo.
        nc.sync.dma_start(xhi[:, 0:8], t["xhi"][:, 0:8])
        # m=0 weights split so the q half lands with the first x chunk
        wt0 = wsp.tile([128, 4, NT_C, 128], F8, tag="wqk", name="wqk")
        nc.sync.dma_start(wt0[:, 0:2], t["wqk"][0][:, 0:2])
        nc.sync.dma_start(xhi[:, 8:16], t["xhi"][:, 8:16])
        nc.sync.dma_start(wt0[:, 2:4], t["wqk"][0][:, 2:4])
        wts_cur = {"wqhi": wt0[:, 0], "wqlo": wt0[:, 1],
                   "wkhi": wt0[:, 2], "wklo": wt0[:, 3]}
        nc.sync.dma_start(xlo[:], t["xlo"][:])
        nc.sync.dma_start(cos2[:], t["cos2"][:])
        nc.sync.dma_start(sin2[:], t["sin2"][:])
        wvhi = xw.tile([128, NT_C, GD], F8, tag="wvhi", name="wvhi")
        wvlo = xw.tile([128, NT_C, GD], F8, tag="wvlo", name="wvlo")
        wv_issued = False

        # q/k-proj: psum [d, t] = sum_c w[c,d].T @ x[c,t], rope after evac
        def qk_mms(ps, wh, wl, tg2, th, terms, start, stop):
            for xs, ws in terms:
                for a in range(NT_C // 2):
                    ts0 = th * 512 + tg2 * 256
                    nc.tensor.matmul(
                        ps[:, tg2 * 256:(tg2 + 1) * 256],
                        ws[:, 2 * a:2 * a + 2, :],
                        xs[:, 2 * a:2 * a + 2, ts0:ts0 + 256],
                        start=start, stop=stop and (xs is terms[-1][0]
                                                    and a == NT_C // 2 - 1),
                        perf_mode=DR)
                    start = False

        def rope(ps, dst, m, th):
            csl = slice(th * 512, (th + 1) * 512)
            qb = rp.tile([128, 512], BF16, tag="qb", name="qb")
            nc.scalar.mul(qb[:], ps[:], 1.0 / WS)
            t1 = rp.tile([128, 512], BF16, tag="t1", name="t1")
            nc.vector.tensor_mul(t1[:], qb[:], cos2[:, csl])
            t2 = rp.tile([128, 512], BF16, tag="t2", name="t2")
            nc.vector.tensor_mul(t2[0:64, :], qb[64:128, :],
                                 sin2[64:128, csl])
            nc.vector.tensor_mul(t2[64:128, :], qb[0:64, :],
                                 sin2[0:64, csl])
            nc.vector.tensor_add(dst[:, m, csl], t1[:], t2[:])

        if True:
            for m in range(NT_HD):
                wts = wts_cur
                if m + 1 < NT_HD:
                    wts_cur = load_qk_w(wsp, m + 1)
                if m == 0:
                    # deferred-xlo emission: all 8 psum groups get their own
                    # bank (a bank can hold only ONE open accumulation group),
                    # xhi terms first, xlo terms + stop once xlo has landed
                    paq0 = tc.alloc_tile_pool(name="paq0", bufs=1,
                                              space="PSUM")
                    pss = {}
                    started = set()
                    # q before k (k weights arrive later), ctile-half-major
                    # so the first groups only need the first xhi DMA chunk
                    for pref in ("wq", "wk"):
                        for ah in range(2):
                            wh, wl = wts[pref + "hi"], wts[pref + "lo"]
                            for th in range(2):
                                for tg2 in range(2):
                                    key = (pref, th, tg2)
                                    if key not in pss:
                                        pss[key] = paq0.tile(
                                            [128, 256], F32,
                                            tag=f"p0{pref}{th}{tg2}",
                                            name="psAq0")
                                    ps = pss[key]
                                    ts0 = th * 512 + tg2 * 256
                                    for xs, ws in ((xhi, wh), (xhi, wl)):
                                        for a in range(ah * 4, ah * 4 + 4):
                                            nc.tensor.matmul(
                                                ps[:],
                                                ws[:, 2 * a:2 * a + 2, :],
                                                xs[:, 2 * a:2 * a + 2,
                                                   ts0:ts0 + 256],
                                                start=(key not in started),
                                                stop=False,
                                                perf_mode=DR)
                                            started.add(key)
                    for pref, dst in (("wq", qr), ("wk", kr)):
                        wh, wl = wts[pref + "hi"], wts[pref + "lo"]
                        for th in range(2):
                            for tg2 in range(2):
                                ps = pss[(pref, th, tg2)]
                                ts0 = th * 512 + tg2 * 256
                                for a in range(NT_C // 2):
                                    nc.tensor.matmul(
                                        ps[:],
                                        wh[:, 2 * a:2 * a + 2, :],
                                        xlo[:, 2 * a:2 * a + 2, ts0:ts0 + 256],
                                        start=False, stop=(a == NT_C // 2 - 1),
                                        perf_mode=DR)
                                csl = slice(ts0, ts0 + 256)
                                qb = rp.tile([128, 256], BF16, tag="qb0",
                                             name="qb0")
                                nc.scalar.mul(qb[:], ps[:], 1.0 / WS)
                                t1 = rp.tile([128, 256], BF16, tag="t10",
                                             name="t10")
                                nc.vector.tensor_mul(t1[:], qb[:],
                                                     cos2[:, csl])
                                t2 = rp.tile([128, 256], BF16, tag="t20",
                                             name="t20")
                                nc.vector.tensor_mul(t2[0:64, :],
                                                     qb[64:128, :],
                                                     sin2[64:128, csl])
                                nc.vector.tensor_mul(t2[64:128, :],
                                                     qb[0:64, :],
                                                     sin2[0:64, csl])
                                nc.vector.tensor_add(dst[:, m, csl],
                                                     t1[:], t2[:])
                    paq0.release()
                    paq = tc.alloc_tile_pool(name="paq", bufs=2, space="PSUM")
                    pav = tc.alloc_tile_pool(name="pav", bufs=2, space="PSUM")
                    continue
                if not wv_issued:
                    # ones (first needed by the first sums matmul in phase B)
                    # and wv queue behind the m=1 weights
                    nc.sync.dma_start(ones[:], t["ones"][:])
                    nc.sync.dma_start(wvhi[:], t["wvhi"][:])
                    nc.sync.dma_start(wvlo[:], t["wvlo"][:])
                    wv_issued = True
                for pref, dst in (("wq", qr), ("wk", kr)):
                    wh, wl = wts[pref + "hi"], wts[pref + "lo"]
                    for th in range(2):
                        ps = paq.tile([128, 512], F32, tag="pq", name="psAq")
                        for tg2 in range(2):
                            qk_mms(ps, wh, wl, tg2, th,
                                   ((xhi, wh), (xhi, wl), (xlo, wh)),
                                   tg2 is not None and True, True)
                        rope(ps, dst, m, th)

        # v-proj: psum [t, hd] = sum_c x[c,t].T @ wv[c,hd], 3-term hi/lo DR
        for tt in range(NT_T):
            tsl = slice(tt * 128, (tt + 1) * 128)
            for half in range(2):
                ps = pav.tile([128, 512], F32, tag="pv", name="psAv")
                for hb2 in range(2):
                    hb = half * 512 + hb2 * 256
                    idx = 0
                    for xs, ws in ((xhi, wvhi), (xhi, wvlo), (xlo, wvhi)):
                        for a in range(NT_C // 2):
                            nc.tensor.matmul(
                                ps[:, hb2 * 256:(hb2 + 1) * 256],
                                xs[:, 2 * a:2 * a + 2, tsl],
                                ws[:, 2 * a:2 * a + 2, hb:hb + 256],
                                start=(idx == 0), stop=(idx == 23),
                                perf_mode=DR)
                            idx += 1
                nc.scalar.mul(vv[:, tt, half * 512:(half + 1) * 512],
                              ps[:], 1.0 / WS)
        pav.release()
        paq.release()

    xw.release()

    wop = tc.alloc_tile_pool(name="wop", bufs=1, side="left")
    wohi = wop.tile([128, NT_HD, C], F8, tag="wohi", name="wohi")
    wolo = wop.tile([128, NT_HD, C], F8, tag="wolo", name="wolo")

    # ---------------- phase B: attention ----------------
    with tc.tile_pool(name="mp", bufs=HPC, side="right") as mp, \
         tc.tile_pool(name="ep", bufs=6, side="right") as ep, \
         tc.tile_pool(name="ptp", bufs=16, side="right") as ptp, \
         tc.tile_pool(name="nrm", bufs=4, side="right") as nrm, \
         tc.tile_pool(name="pcs", bufs=4, space="PSUM") as pcs, \
         tc.tile_pool(name="pca", bufs=2, space="PSUM") as pca, \
         tc.tile_pool(name="pcm", bufs=2, space="PSUM") as pcm:
        # all expb masks up front, then the (big) out-proj weights, so the
        # per-head mask is never behind a 4MB transfer on the DMA queue
        expbs = []
        for h in range(HPC):
            eb = mp.tile([128, EW], BF16, tag="expb", name="expb")
            nc.sync.dma_start(eb[:], t["expb"][h])
            expbs.append(eb)
        nc.sync.dma_start(wohi[:], t["wohi"][:])
        nc.sync.dma_start(wolo[:], t["wolo"][:])
        # software pipeline: emit group k's scores/exp/mask one group ahead
        # of group k-1's PV+sums, so the in-order PE never sits in the
        # scores->exp->mask->PV latency chain (worst for the 2-tile group 0)
        def emit_sc(h, gi):
            """scores matmuls + exp; masks are emitted separately so the
            previous group's normalize is not queued behind them on DVE"""
            i0 = gi * QG
            js = jtiles(i0, h)
            scs = []
            for pi in range(0, len(js), 2):
                pair = js[pi:pi + 2]
                npc = len(pair)
                s_ps = pcs.tile([128, 512], F32, tag="s", name="s_ps")
                for k2, j0 in enumerate(pair):
                    nc.tensor.matmul(
                        s_ps[:, k2 * 256:(k2 + 1) * 256],
                        kr[:, h, j0:j0 + 128],
                        qr[:, h, i0:i0 + QG],
                        start=True, stop=True)
                e = ep.tile([128, 512], BF16, tag="e", name="e")
                nc.scalar.activation(e[:, 0:npc * 256], s_ps[:, 0:npc * 256],
                                     AF.Exp, scale=SCALE)
                scs.append((pair, e))
            return scs

        def emit_mk(h, gi, scs):
            i0 = gi * QG
            expb = expbs[h]
            pts = []
            for pair, e in scs:
                for k2, j0 in enumerate(pair):
                    soff = MC0 - (j0 - i0)
                    pT = ptp.tile([128, QG], BF16, tag="pT", name="pT")
                    # spill a fraction of the mask-muls to GPSIMD to keep
                    # DVE off the phase-B critical path
                    self_cnt = emit_mk.cnt = getattr(emit_mk, 'cnt', 0) + 1
                    eng = nc.gpsimd if self_cnt % 7 == 6 else nc.vector
                    eng.tensor_mul(
                        pT[:], e[:, k2 * 256:(k2 + 1) * 256],
                        expb[:, soff:soff + QG])
                    pts.append((j0, pT))
            return pts

        def emit_pv(h, gi, pts):
            i0 = gi * QG
            nj = len(pts)
            attn_ps = pca.tile([128, QG], F32, tag="attn", name="attn_ps")
            sums_ps = pcm.tile([128, QG], F32, tag="sums", name="sums_ps")
            for idx, (j0, pT) in enumerate(pts):
                nc.tensor.matmul(
                    attn_ps[:],
                    vv[:, j0 // 128, h * 128:(h + 1) * 128],
                    pT[:],
                    start=(idx == 0), stop=(idx == nj - 1))
                nc.tensor.matmul(
                    sums_ps[:],
                    ones[:],
                    pT[:],
                    start=(idx == 0), stop=(idx == nj - 1))
            tmp = nrm.tile([128, QG], BF16, tag="tmp", name="tmp")
            rec = nrm.tile([128, QG], F32, tag="rec", name="rec")
            nc.vector.reciprocal(rec[:], sums_ps[:])
            nc.vector.tensor_mul(tmp[:], attn_ps[:], rec[:])
            # hi/lo split spread over GPSIMD + Act (both off critical path)
            if gi % 2 == 0:
                nc.gpsimd.tensor_copy(ahi[:, h, i0:i0 + QG], tmp[:])
            else:
                nc.scalar.copy(ahi[:, h, i0:i0 + QG], tmp[:])
            nc.gpsimd.tensor_sub(alo[:, h, i0:i0 + QG], tmp[:],
                                 ahi[:, h, i0:i0 + QG])

        groups = [(h, gi) for h in range(HPC) for gi in range(NQG)]
        prev = None
        for h, gi in groups:
            scs = emit_sc(h, gi)
            pts = emit_mk(h, gi, scs)
            if prev is not None:
                emit_pv(*prev)
            prev = (h, gi, pts)
        emit_pv(*prev)

    # ---------------- phase C: out-proj (3-term hi/lo DR) ----------------
    with tc.tile_pool(name="og", bufs=4, side="right") as og, \
         tc.tile_pool(name="pd", bufs=4, space="PSUM") as pd:
        for tt in range(NT_T):
            tsl = slice(tt * 128, (tt + 1) * 128)
            for cq in range(2):
                o = og.tile([128, 1024], BF16, tag="o", name="o")
                for cb4 in range(4):
                    cb = cq * 4 + cb4
                    ps = pd.tile([128, 256], F32, tag="po", name="psD")
                    idx = 0
                    for a_, w_ in ((ahi, wohi), (alo, wohi), (ahi, wolo)):
                        for hp in range(NT_HD // 2):
                            nc.tensor.matmul(
                                ps[:],
                                a_[:, 2 * hp:2 * hp + 2, tsl],
                                w_[:, 2 * hp:2 * hp + 2,
                                   cb * 256:(cb + 1) * 256],
                                start=(idx == 0), stop=(idx == 11),
                                perf_mode=DR)
                            idx += 1
                    osl = o[:, cb4 * 256:(cb4 + 1) * 256]
                    if cb % 2 == 0:
                        nc.scalar.mul(osl, ps[:], 1.0 / WS)
                    else:
                        nc.vector.tensor_scalar_mul(osl, ps[:], 1.0 / WS)
                nc.sync.dma_start(
                    t["out"][tt * 128:(tt + 1) * 128,
                             cq * 1024:(cq + 1) * 1024], o[:])

    wop.release()
    pp.release()
    cpool.release()


def build_nc(enable_asserts=False, reps=1):
    nc = bacc.Bacc("TRN2", target_bir_lowering=False, debug=False,
                   enable_asserts=enable_asserts, num_devices=8)
    t = {}
    t["xhi"] = nc.dram_tensor("xhi", [128, NT_C, L], F8, kind="ExternalInput").ap()
    t["xlo"] = nc.dram_tensor("xlo", [128, NT_C, L], F8, kind="ExternalInput").ap()
    t["wvhi"] = nc.dram_tensor("wvhi", [128, NT_C, GD], F8, kind="ExternalInput").ap()
    t["wvlo"] = nc.dram_tensor("wvlo", [128, NT_C, GD], F8, kind="ExternalInput").ap()
    t["wqk"] = nc.dram_tensor("wqk", [NT_HD, 128, 4, NT_C, 128], F8,
                              kind="ExternalInput").ap()
    t["wohi"] = nc.dram_tensor("wohi", [128, NT_HD, C], F8, kind="ExternalInput").ap()
    t["wolo"] = nc.dram_tensor("wolo", [128, NT_HD, C], F8, kind="ExternalInput").ap()
    t["cos2"] = nc.dram_tensor("cos2", [128, L], BF16, kind="ExternalInput").ap()
    t["sin2"] = nc.dram_tensor("sin2", [128, L], BF16, kind="ExternalInput").ap()
    t["expb"] = nc.dram_tensor("expb", [HPC, 128, EW], BF16, kind="ExternalInput").ap()
    t["ones"] = nc.dram_tensor("ones", [128, 128], BF16, kind="ExternalInput").ap()
    t["out"] = nc.dram_tensor("out", [L, C], BF16, kind="ExternalOutput").ap()
    with tile.TileContext(nc) as tc:
        for _ in range(reps):
            emit(tc, t)
    nc.compile()
    return nc


def _split8(a):
    """hi/lo fp8 split of an fp32 array."""
    f8 = ml_dtypes.float8_e4m3
    hi = a.astype(f8)
    lo = (a - hi.astype(np.float32)).astype(f8)
    return hi, lo


def marshal(inputs):
    x = np.asarray(inputs["x"], np.float32)
    wq = np.asarray(inputs["wq"], np.float32)
    wkv = np.asarray(inputs["wkv"], np.float32)
    wo = np.asarray(inputs["wo"], np.float32)
    alibi = np.asarray(inputs["alibi_slopes"], np.float32)
    wk_full, wv_full = wkv[:C], wkv[C:]

    perm = np.concatenate([np.arange(0, D, 2), np.arange(1, D, 2)])
    head_perm = np.concatenate([h * D + perm for h in range(H)])
    wq_p, wk_p = wq[head_perm], wk_full[head_perm]

    t_abs = np.arange(W, W + L, dtype=np.float64)
    inv = 1.0 / (10000.0 ** (np.arange(0, D, 2, dtype=np.float64) / D))
    fr = np.outer(t_abs, inv)
    cosT = np.cos(fr).T.astype(np.float32)
    sinT = np.sin(fr).T.astype(np.float32)
    bf = ml_dtypes.bfloat16
    cos2 = np.ascontiguousarray(np.concatenate([cosT, cosT], 0)).astype(bf)
    # partition-swapped sin master: rows 0:64 = +sinT (mult for x1 -> out
    # rows 64:128), rows 64:128 = -sinT (mult for x2 -> out rows 0:64);
    # keeps both tensor_tensor inputs at the same base partition.
    sin2 = np.ascontiguousarray(np.concatenate([sinT, -sinT], 0)).astype(bf)

    # expb master: [dj, y] = exp(slope*rel) * window, rel = dj - y + MC0
    dj = np.arange(128)[:, None]
    y = np.arange(EW)[None, :]
    rel = (dj - y + MC0).astype(np.float64)
    win = (rel <= 0) & (rel >= -W)

    in_maps = []
    for core in range(8):
        b, g = divmod(core, 2)
        hsel = CORE_HEADS(g)
        rsel = np.concatenate([np.arange(hh * D, (hh + 1) * D) for hh in hsel])
        xb = x[:, b, :]                                   # (L, C)
        xT = np.ascontiguousarray(xb.T).reshape(NT_C, 128, L)
        xT = np.ascontiguousarray(xT.transpose(1, 0, 2))  # [128, NT_C, L]
        xhi, xlo = _split8(xT)
        # wv: [c-part, ctile, hd]
        wv_m = np.ascontiguousarray(
            wv_full[rsel].T.reshape(NT_C, 128, GD).transpose(1, 0, 2))
        wvhi, wvlo = _split8(wv_m * WS)
        # wq/wk: [m, c-part, ctile, d]
        def qk_m(w):
            wg = w[rsel].reshape(NT_HD, 128, NT_C, 128)   # [m, d, ct, cp]
            return np.ascontiguousarray(wg.transpose(0, 3, 2, 1))
        wqhi, wqlo = _split8(qk_m(wq_p) * WS)
        wkhi, wklo = _split8(qk_m(wk_p) * WS)
        wqk = np.ascontiguousarray(
            np.stack([wqhi, wqlo, wkhi, wklo], axis=2))
        # wo: [dv-part, hdtile, c]
        wo_m = np.ascontiguousarray(
            wo[:, rsel].T.reshape(NT_HD, 128, C).transpose(1, 0, 2))
        wohi, wolo = _split8(wo_m * WS)
        expb = np.zeros((HPC, 128, EW), bf)
        for hh in range(HPC):
            s = float(alibi[hsel[hh]])
            expb[hh] = np.where(win, np.exp(s * rel), 0.0).astype(bf)
        in_maps.append(dict(
            xhi=xhi, xlo=xlo, wvhi=wvhi, wvlo=wvlo, wqk=wqk,
            wohi=wohi, wolo=wolo,
            cos2=cos2, sin2=sin2, expb=expb,
            ones=np.ones((128, 128), bf)))
    return in_maps


def gather(results, bo):
    bo = np.asarray(bo, np.float32)
    out = np.empty((L, N, C), np.float32)
    for b in range(N):
        out[:, b, :] = (results[2 * b]["out"].astype(np.float32)
                        + results[2 * b + 1]["out"].astype(np.float32)
                        + bo[None, :])
    return out


_NC_CACHE = {}


def _get_nc():
    if "nc" not in _NC_CACHE:
        _NC_CACHE["nc"] = build_nc()
    return _NC_CACHE["nc"]


def kernel(**inputs):
    from concourse import bass_utils
    nc = _get_nc()
    in_maps = marshal(inputs)
    res = bass_utils.run_bass_kernel_spmd(nc, in_maps, core_ids=list(range(8)))
    return gather(res.results, inputs["bo"])
